# revision 21
# baseline (speedup 1.0000x reference)
"""Trainium2 Bass kernel for nn_Attention (dense transformer block attention).

Full-input contract: kernel(**inputs) takes the unsharded numpy inputs and
returns the full (4096, 768) float32 output.

Sharding: 12 heads x 4096 queries -> 4 head-groups (3 heads) x 2 query
halves (2048 queries) = 8 NeuronCores. Each core computes K/V for its 3
heads over all tokens, attention for its 2048 queries, and a partial
output projection (its 192 channels of the contraction); the host sums
the 4 head-group partials per query half and adds the bias.

Per-core token order is permuted so the core's own queries are always
columns 0:2048 of xw (softmax is permutation-invariant over keys), which
removes the separate qx input.

Score matmuls are emitted as row-tiled pairs (two 64-contraction matmuls
at row groups 0/64 in adjacent instructions) so the PE array runs them
concurrently: heads 0/1 pair with each other, head 2 pairs its two
token-half layouts.
"""

import os

import numpy as np
import ml_dtypes

import concourse.bass as bass
import concourse.tile as tile
from concourse import bacc, mybir
from concourse import bass_utils

BF16 = mybir.dt.bfloat16
F32 = mybir.dt.float32
NPBF16 = ml_dtypes.bfloat16

N_TOK = 4096
DIM = 768
H = 12
HD = 64
N_CORES = 8
HPC = 3              # heads per core (head-group size)
NQ = N_TOK // 2      # queries per core
CCH = DIM // 128     # 6 contraction chunks
TCH = N_TOK // 512   # 8 token chunks of 512
KCH = N_TOK // 128   # 32 key chunks of 128
QB = NQ // 512       # 4 query blocks per core
SWEEP = 3            # exp jobs per sweep (3 PSUM banks)
SCALE = HD ** -0.5

_cache = {}


def _build_program(repeat=1):
    nc = bacc.Bacc(
        "TRN2",
        target_bir_lowering=False,
        debug=False,
        enable_asserts=False,
        num_devices=N_CORES,
    )

    xw_d = nc.dram_tensor("xw", [128, CCH, N_TOK], BF16, kind="ExternalInput").ap()
    wk_d = nc.dram_tensor("wk", [128, CCH, HPC * HD], BF16, kind="ExternalInput").ap()
    wq_d = nc.dram_tensor("wq", [128, CCH, HPC * HD], BF16, kind="ExternalInput").ap()
    wv_d = nc.dram_tensor("wv", [128, CCH, HPC * HD], BF16, kind="ExternalInput").ap()
    wp_d = nc.dram_tensor("wp", [HPC, HD, DIM], BF16, kind="ExternalInput").ap()
    out_d = nc.dram_tensor("outp", [DIM, NQ], BF16, kind="ExternalOutput").ap()

    with tile.TileContext(nc) as tc:
        with (
            tc.tile_pool(name="persist", bufs=1) as pp,
            tc.tile_pool(name="psS", bufs=2, space="PSUM") as psS,
            tc.tile_pool(name="pvp", bufs=2, space="PSUM") as pvp,
            tc.tile_pool(name="expp", bufs=12) as expp,
            tc.tile_pool(name="nrm", bufs=4) as nrm,
            tc.tile_pool(name="accp", bufs=3) as accp,
            tc.tile_pool(name="nrmd", bufs=3, space="DRAM") as nrmd,
            tc.tile_pool(name="outs", bufs=3) as outs,
        ):
            xw = pp.tile([128, CCH, N_TOK], BF16, tag="xw")
            wk = pp.tile([128, CCH, HPC * HD], BF16, tag="wk")
            wq = pp.tile([128, CCH, HPC * HD], BF16, tag="wq")
            wv = pp.tile([128, CCH, HPC * HD], BF16, tag="wv")
            wp01 = pp.tile([128, DIM], BF16, tag="wp01")
            wp2 = pp.tile([HD, DIM], BF16, tag="wp2")
            kT01 = pp.tile([128, N_TOK], BF16, tag="kT01")  # heads 0,1 stacked
            kT2ab = pp.tile([128, N_TOK // 2], BF16, tag="kT2ab")  # head 2
            qT01 = pp.tile([128, NQ], BF16, tag="qT01")
            qT2ab = pp.tile([128, NQ], BF16, tag="qT2ab")
            # V in token-major layout with a ones column appended (col HD)
            v2 = [
                pp.tile([128, KCH, HD + 1], BF16, tag=f"v2_{h}", name=f"v2_{h}")
                for h in range(HPC)
            ]
            # attention outputs: heads 0/1 stacked on partitions, head 2 alone
            attT2 = pp.tile([128, NQ], BF16, tag="attT2")
            attTh2 = pp.tile([HD, NQ], BF16, tag="attTh2")
            for h in range(HPC):
                nc.vector.memset(v2[h][:, :, HD : HD + 1], 1.0)
            onesb = pp.tile([HD + 1, HD], BF16, tag="onesb")
            nc.vector.memset(onesb, 1.0)
            ones_k = pp.tile([128, 1], BF16, tag="ones_k")
            nc.vector.memset(ones_k, 1.0)
            # warm the ACT exp table set before the first real sweep
            warm = pp.tile([1, 2], F32, tag="warm")
            nc.vector.memset(warm, 0.0)
            nc.scalar.activation(
                out=warm, in_=warm, func=mybir.ActivationFunctionType.Exp
            )

            counter = [0]

            def uname(p):
                counter[0] += 1
                return f"{p}{counter[0]}"

            def mm_slot():
                return psS.tile(
                    [128, SWEEP * 512], F32, tag="scores", name=uname("mmslot")
                )

            def qkv_group(w_sb, m_sl, x_sb, n_sl, out_sb, out_sl, m_par):
                ps = mm_slot()
                for c in range(CCH):
                    nc.tensor.matmul(
                        ps[0:m_par, 0:512],
                        w_sb[:, c, m_sl],
                        x_sb[:, c, n_sl],
                        start=(c == 0),
                        stop=(c == CCH - 1),
                    )
                nc.vector.tensor_copy(out=out_sb[:, out_sl], in_=ps[0:m_par, 0:512])

            def v2_group(tt, heads=(0, 1, 2)):
                tsl = slice(128 * tt, 128 * (tt + 1))
                msl = slice(HD * heads[0], HD * (heads[-1] + 1))
                ps = mm_slot()
                for c in range(CCH):
                    nc.tensor.matmul(
                        ps[:, 0 : HD * len(heads)],
                        xw[:, c, tsl],
                        wv[:, c, msl],
                        start=(c == 0),
                        stop=(c == CCH - 1),
                    )
                for i, h in enumerate(heads):
                    nc.vector.tensor_copy(
                        out=v2[h][:, tt, 0:HD],
                        in_=ps[:, HD * i : HD * (i + 1)],
                    )

            def finish_unit(h, qb, pv, fast_tail=False):
                qsl = slice(512 * qb, 512 * (qb + 1))
                pvs = nrm.tile([HD + 1, 512], F32, tag="pvs", name=uname("pvs"))
                rec = nrm.tile([HD + 1, 512], F32, tag="rec", name=uname("rec"))
                nc.vector.reciprocal(out=rec[HD : HD + 1, :], in_=pv[HD : HD + 1, :])
                nc.vector.tensor_copy(out=pvs, in_=pv)
                bc = nrm.tile([HD, 512], F32, tag="bc", name=uname("bc"))
                if fast_tail:
                    recb = nrm.tile(
                        [HD + 1, 512], BF16, tag="recb", name=uname("recb")
                    )
                    nc.vector.tensor_copy(
                        out=recb[HD : HD + 1, :], in_=rec[HD : HD + 1, :]
                    )
                    bcp = mm_slot()
                    nc.tensor.matmul(
                        bcp[0:HD, 0:512],
                        onesb[HD : HD + 1, :],
                        recb[HD : HD + 1, :],
                        start=True,
                        stop=True,
                    )
                    nc.vector.tensor_copy(out=bc, in_=bcp[0:HD, 0:512])
                else:
                    recd = nrmd.tile([1, 512], F32, tag="recd", name=uname("recd"))
                    nc.sync.dma_start(out=recd, in_=rec[HD : HD + 1, :])
                    nc.sync.dma_start(out=bc, in_=recd[:].to_broadcast((HD, 512)))
                if h < 2:
                    nc.vector.tensor_mul(
                        attT2[HD * h : HD * (h + 1), qsl], pvs[0:HD, :], bc
                    )
                else:
                    nc.vector.tensor_mul(attTh2[:, qsl], pvs[0:HD, :], bc)

            def emit_body():
                # Load order: first weights + x chunks feeding kT01/qT01, so
                # the first matmul can start ~3us in instead of one bulk DMA.
                nc.sync.dma_start(out=wk, in_=wk_d)
                nc.sync.dma_start(out=xw[:, 0:3, 0:512], in_=xw_d[:, 0:3, 0:512])
                nc.sync.dma_start(out=xw[:, 3:6, 0:512], in_=xw_d[:, 3:6, 0:512])
                nc.sync.dma_start(out=wq, in_=wq_d)
                nc.sync.dma_start(out=wv, in_=wv_d)
                for t in range(1, TCH):
                    sl = slice(512 * t, 512 * (t + 1))
                    nc.sync.dma_start(out=xw[:, :, sl], in_=xw_d[:, :, sl])
                nc.sync.dma_start(out=wp01[0:HD, :], in_=wp_d[0])
                nc.sync.dma_start(out=wp01[HD:128, :], in_=wp_d[1])
                nc.sync.dma_start(out=wp2, in_=wp_d[2])

                kT01_pending = list(range(TCH))
                qT0_pending = [0]

                def kq_jit(tokens_needed):
                    while kT01_pending and 512 * kT01_pending[0] < tokens_needed:
                        t = kT01_pending.pop(0)
                        sl = slice(512 * t, 512 * (t + 1))
                        qkv_group(wk, slice(0, 128), xw, sl, kT01, sl, 128)
                        if qT0_pending:
                            qT0_pending.pop()
                            qkv_group(
                                wq,
                                slice(0, 128),
                                xw,
                                slice(0, 512),
                                qT01,
                                slice(0, 512),
                                128,
                            )

                v2_pending = list(range(KCH))
                v2_done = [0]

                def v2_step():
                    # first unit only needs heads 0/1; head-2 V is queued as
                    # background work for the following unit.
                    if v2_pending:
                        v2_group(v2_pending.pop(0), heads=(0, 1))
                        v2_done[0] += 1

                background = []

                def kt2_pair(p):
                    # col-tiled concurrent pair: rows 0:63 <- tokens 1024p..,
                    # rows 64:127 <- +512; interleaved c-loop so the two
                    # column-group matmuls sit adjacent and overlap.
                    ps = mm_slot()
                    sl_a = slice(1024 * p, 1024 * p + 512)
                    sl_b = slice(1024 * p + 512, 1024 * p + 1024)
                    for c in range(CCH):
                        nc.tensor.matmul(
                            ps[0:HD, 0:512],
                            wk[:, c, 128 : 128 + HD],
                            xw[:, c, sl_a],
                            start=(c == 0),
                            stop=(c == CCH - 1),
                            skip_group_check=True,
                        )
                        nc.tensor.matmul(
                            ps[HD:128, 0:512],
                            wk[:, c, 128 : 128 + HD],
                            xw[:, c, sl_b],
                            start=(c == 0),
                            stop=(c == CCH - 1),
                            skip_group_check=True,
                        )
                    nc.vector.tensor_copy(
                        out=kT2ab[:, 512 * p : 512 * (p + 1)], in_=ps[:, 0:512]
                    )

                def qt2_fill(t):
                    sl = slice(512 * t, 512 * (t + 1))
                    ps = mm_slot()
                    for c in range(CCH):
                        nc.tensor.matmul(
                            ps[0:HD, 0:512],
                            wq[:, c, 128 : 128 + HD],
                            xw[:, c, sl],
                            start=(c == 0),
                            stop=(c == CCH - 1),
                        )
                    nc.vector.tensor_copy(out=qT2ab[0:HD, sl], in_=ps[0:HD, 0:512])

                sweep_no = [0]

                def tick_background():
                    # chunky items (kt2/qt2/proj bursts) borrow a psS slot;
                    # pace them so consecutive items can't grab both slots
                    # and stall the exp pipeline.
                    sweep_no[0] += 1
                    if background and sweep_no[0] % 2 == 0:
                        background.pop(0)()

                def drain_background():
                    while background:
                        background.pop(0)()

                def emit_pv(pv, ex, sweep_jobs, pv_start, pv_stop):
                    """unit2 path: sweep_jobs is a list of (slot_j, 2, kc);
                    pv[2] is a [HD+1, 512] bank using the ones-column trick."""
                    for j, key, kc in sweep_jobs:
                        nc.tensor.matmul(
                            pv[key],
                            v2[key][:, kc, :],
                            ex[:, 512 * j : 512 * (j + 1)],
                            start=pv_start[key],
                            stop=(kc == pv_stop[key]),
                            skip_group_check=True,
                        )
                        pv_start[key] = False

                def emit_pv01(pv, acc, ex, sweep_jobs, pv_start):
                    """unit01 path: col-tiled PV pair (h0 -> pv rows 0:64,
                    h1 -> rows 64:128, both M=64, concurrent col groups) plus
                    DVE bf16 accumulation of the softmax denominators."""
                    for j, h, kc in sweep_jobs:
                        exs = ex[:, 512 * j : 512 * (j + 1)]
                        nc.tensor.matmul(
                            pv[HD * h : HD * (h + 1), :],
                            v2[h][:, kc, 0:HD],
                            exs,
                            start=pv_start[h],
                            stop=(kc == KCH - 1),
                            skip_group_check=True,
                        )
                        pv_start[h] = False
                        if kc == 0:
                            nc.vector.tensor_copy(out=acc[h], in_=exs)
                        else:
                            nc.vector.tensor_add(acc[h], acc[h], exs)

                def finish_pair(qb, pv, acc):
                    """Normalize both heads of a unit01: partition-reduce the
                    bf16 denominator partials with a ones matmul, reciprocal,
                    broadcast back across partitions with col-tiled outer
                    products, then scale. PSUM scratch comes from pvp banks
                    so the psS sweep rotation is never blocked."""
                    qsl = slice(512 * qb, 512 * (qb + 1))
                    # copy pv out first: it frees pv's bank before den/bc
                    # rotate into it (emitting it later would deadlock the
                    # DVE FIFO against the slot-reuse WAR dependency).
                    pvs = nrm.tile([128, 512], F32, tag="pvs", name=uname("pvs"))
                    nc.vector.tensor_copy(out=pvs, in_=pv)
                    dens = [
                        pvp.tile([1, 512], F32, tag="pv", name=uname("den"))
                        for _ in range(2)
                    ]
                    bc = pvp.tile([128, 512], F32, tag="pv", name=uname("bc"))
                    recbs = []
                    for h in range(2):
                        nc.tensor.matmul(
                            dens[h], ones_k, acc[h], start=True, stop=True
                        )
                        rec = nrm.tile([1, 512], F32, tag="rec", name=uname("rec"))
                        nc.vector.reciprocal(out=rec, in_=dens[h])
                        recb = nrm.tile(
                            [1, 512], BF16, tag="recb", name=uname("recb")
                        )
                        nc.vector.tensor_copy(out=recb, in_=rec)
                        recbs.append(recb)
                    for h in range(2):
                        nc.tensor.matmul(
                            bc[HD * h : HD * (h + 1), :],
                            onesb[0:1, 0:HD],
                            recbs[h],
                            start=True,
                            stop=True,
                            skip_group_check=True,
                        )
                    nc.vector.tensor_mul(attT2[:, qsl], pvs, bc)

                def unit01(qb, first=False):
                    """Heads 0+1 for one query block: row-tiled score pairs,
                    col-tiled PV pairs, DVE denominator accumulation."""
                    qsl = slice(512 * qb, 512 * (qb + 1))
                    pv = pvp.tile([128, 512], F32, tag="pv", name=uname("pv"))
                    acc = [
                        accp.tile([128, 512], BF16, tag="acc", name=uname("acc"))
                        for _ in range(2)
                    ]
                    pv_start = {0: True, 1: True}
                    jobs = [(h, kc) for kc in range(KCH) for h in range(2)]
                    deferred = []
                    ji = 0
                    while ji < len(jobs):
                        nch = min(SWEEP, len(jobs) - ji)
                        sw = jobs[ji : ji + nch]
                        kq_jit(128 * (max(kc for _, kc in sw) + 1))
                        ps = mm_slot()
                        ex = expp.tile(
                            [128, SWEEP * 512], BF16, tag="ex", name=uname("ex")
                        )
                        for j, (h, kc) in enumerate(sw):
                            nc.tensor.matmul(
                                ps[:, 512 * j : 512 * (j + 1)],
                                kT01[HD * h : HD * (h + 1), 128 * kc : 128 * (kc + 1)],
                                qT01[HD * h : HD * (h + 1), qsl],
                                start=True,
                                stop=True,
                            )
                        nc.scalar.activation(
                            out=ex[:, 0 : 512 * nch],
                            in_=ps[:, 0 : 512 * nch],
                            func=mybir.ActivationFunctionType.Exp,
                            scale=float(SCALE),
                        )
                        sweep_jobs = [(j, h, kc) for j, (h, kc) in enumerate(sw)]
                        ji += nch
                        if first:
                            if v2_pending:
                                for _ in range(2):
                                    v2_step()
                            else:
                                tick_background()
                            deferred.append((ex, sweep_jobs))
                            while deferred and (
                                max(kc for _, _, kc in deferred[0][1])
                                < v2_done[0]
                                or len(deferred) >= 11
                            ):
                                dex, dsw = deferred.pop(0)
                                while v2_done[0] <= max(kc for _, _, kc in dsw):
                                    v2_step()
                                emit_pv01(pv, acc, dex, dsw, pv_start)
                        else:
                            emit_pv01(pv, acc, ex, sweep_jobs, pv_start)
                            tick_background()
                    if first:
                        while v2_pending:
                            v2_step()
                        for dex, dsw in deferred:
                            emit_pv01(pv, acc, dex, dsw, pv_start)
                        kq_jit(N_TOK + 1)
                    finish_pair(qb, pv, acc)

                def unit2(qb, fast_tail=False):
                    """Head 2 for one query block: the two token-half layouts
                    of kT2ab/qT2ab pair as concurrent row tiles."""
                    qsl = slice(512 * qb, 512 * (qb + 1))
                    pv = {2: pvp.tile([HD + 1, 512], F32, tag="pv", name=uname("pv"))}
                    pv_start = {2: True}
                    jobs = []
                    for idx in range(KCH // 2):
                        for half in range(2):
                            p_, i_ = idx // 4, idx % 4
                            kcg = 8 * p_ + 4 * half + i_
                            jobs.append((half, p_, i_, kcg))
                    pv_stop = {2: jobs[-1][3]}
                    ji = 0
                    while ji < len(jobs):
                        nch = min(SWEEP, len(jobs) - ji)
                        sw = jobs[ji : ji + nch]
                        ps = mm_slot()
                        ex = expp.tile(
                            [128, SWEEP * 512], BF16, tag="ex", name=uname("ex")
                        )
                        for j, (half, p_, i_, kcg) in enumerate(sw):
                            nc.tensor.matmul(
                                ps[:, 512 * j : 512 * (j + 1)],
                                kT2ab[
                                    HD * half : HD * (half + 1),
                                    512 * p_ + 128 * i_ : 512 * p_ + 128 * (i_ + 1),
                                ],
                                qT2ab[HD * half : HD * (half + 1), qsl],
                                start=True,
                                stop=True,
                            )
                        nc.scalar.activation(
                            out=ex[:, 0 : 512 * nch],
                            in_=ps[:, 0 : 512 * nch],
                            func=mybir.ActivationFunctionType.Exp,
                            scale=float(SCALE),
                        )
                        sweep_jobs = [
                            (j, 2, kcg) for j, (_, _, _, kcg) in enumerate(sw)
                        ]
                        emit_pv(pv, ex, sweep_jobs, pv_start, pv_stop)
                        ji += nch
                        tick_background()
                    finish_unit(2, qb, pv[2], fast_tail=fast_tail)

                def proj_burst(qb, half):
                    """Three 128-channel slices of the output projection in
                    one borrowed psS tile. h0+h1 contract in a single K=128
                    matmul via the stacked attT2 layout; h2 adds K=64."""
                    qsl = slice(512 * qb, 512 * (qb + 1))
                    ps = mm_slot()
                    for i in range(SWEEP):
                        m = SWEEP * half + i
                        pj = ps[:, 512 * i : 512 * (i + 1)]
                        nc.tensor.matmul(
                            pj,
                            wp01[:, 128 * m : 128 * (m + 1)],
                            attT2[:, qsl],
                            start=True,
                            stop=False,
                        )
                        nc.tensor.matmul(
                            pj,
                            wp2[:, 128 * m : 128 * (m + 1)],
                            attTh2[:, qsl],
                            start=False,
                            stop=True,
                        )
                    ot = outs.tile([128, SWEEP, 512], BF16, tag="ot", name=uname("ot"))
                    for i in range(SWEEP):
                        nc.vector.tensor_copy(
                            out=ot[:, i, :], in_=ps[:, 512 * i : 512 * (i + 1)]
                        )
                    m0 = SWEEP * half
                    nc.sync.dma_start(
                        out=out_d[128 * m0 : 128 * (m0 + SWEEP), qsl].rearrange(
                            "(a p) n -> p a n", p=128
                        ),
                        in_=ot,
                    )

                def proj_last(qb):
                    """Final query block: pack all 6 slices into psS tiles
                    (no sweeps follow, banks are free)."""
                    qsl = slice(512 * qb, 512 * (qb + 1))
                    sA = mm_slot()
                    sB = mm_slot()
                    pjs = [sA[:, 512 * i : 512 * (i + 1)] for i in range(SWEEP)]
                    pjs += [sB[:, 512 * i : 512 * (i + 1)] for i in range(SWEEP)]
                    for m in range(DIM // 128):
                        nc.tensor.matmul(
                            pjs[m],
                            wp01[:, 128 * m : 128 * (m + 1)],
                            attT2[:, qsl],
                            start=True,
                            stop=False,
                        )
                        nc.tensor.matmul(
                            pjs[m],
                            wp2[:, 128 * m : 128 * (m + 1)],
                            attTh2[:, qsl],
                            start=False,
                            stop=True,
                        )
                    for mp in range(DIM // 256):
                        ot = outs.tile(
                            [128, 2, 512], BF16, tag="ot2", name=uname("ot")
                        )
                        nc.vector.tensor_copy(out=ot[:, 0, :], in_=pjs[2 * mp])
                        nc.scalar.copy(out=ot[:, 1, :], in_=pjs[2 * mp + 1])
                        nc.sync.dma_start(
                            out=out_d[256 * mp : 256 * (mp + 1), qsl].rearrange(
                                "(a p) n -> p a n", p=128
                            ),
                            in_=ot,
                        )

                # qT01 for qb 1-3 must be ready before their units; head-2
                # K/Q (kt2/qt2) before unit2(0).
                def qt01_fill(t):
                    sl = slice(512 * t, 512 * (t + 1))
                    qkv_group(wq, slice(0, 128), xw, sl, qT01, sl, 128)

                def v2h2_quad(tt0):
                    # head-2 V for 4 key chunks in one borrowed psS bank:
                    # sequential accumulation groups, plain per-chunk copies.
                    ps = mm_slot()
                    for j in range(4):
                        for c in range(CCH):
                            nc.tensor.matmul(
                                ps[:, HD * j : HD * (j + 1)],
                                xw[:, c, 128 * (tt0 + j) : 128 * (tt0 + j + 1)],
                                wv[:, c, 2 * HD : 3 * HD],
                                start=(c == 0),
                                stop=(c == CCH - 1),
                                skip_group_check=True,
                            )
                    for j in range(4):
                        nc.vector.tensor_copy(
                            out=v2[2][:, tt0 + j, 0:HD],
                            in_=ps[:, HD * j : HD * (j + 1)],
                        )

                for t in range(1, QB):
                    background.append(lambda t=t: qt01_fill(t))
                unit01(0, first=True)
                drain_background()
                for tt0 in range(0, KCH, 4):
                    background.append(lambda tt0=tt0: v2h2_quad(tt0))
                for p in range(TCH // 2):
                    background.append(lambda p=p: kt2_pair(p))
                for t in range(QB):
                    background.append(lambda t=t: qt2_fill(t))
                background.append(
                    lambda: nc.sync.dma_start(
                        out=qT2ab[HD:128, :], in_=qT2ab[0:HD, :]
                    )
                )
                unit01(1)
                drain_background()
                unit2(0)
                background.append(lambda: proj_burst(0, 0))
                background.append(lambda: proj_burst(0, 1))
                unit01(2)
                unit2(1)
                background.append(lambda: proj_burst(1, 0))
                background.append(lambda: proj_burst(1, 1))
                unit01(3)
                unit2(2)
                background.append(lambda: proj_burst(2, 0))
                background.append(lambda: proj_burst(2, 1))
                unit2(3, fast_tail=True)
                drain_background()
                proj_last(3)

            for _rep in range(repeat):
                emit_body()

    nc.compile()
    return nc


def _prep_contraction_major(a_t: np.ndarray) -> np.ndarray:
    """(DIM, n) array -> (128, CCH, n) bf16 in SBUF layout."""
    n = a_t.shape[1]
    return np.ascontiguousarray(
        a_t.reshape(CCH, 128, n).transpose(1, 0, 2).astype(NPBF16)
    )


def prep_in_maps(x, w_qkv, w_proj):
    """Build the 8 per-core input dicts from the full-precision inputs."""
    # Token permutation per query half: own queries first.
    xw_np = [
        _prep_contraction_major(
            np.ascontiguousarray(
                (x if s == 0 else np.concatenate([x[NQ:], x[:NQ]])).T
            )
        )
        for s in range(2)
    ]

    wk_np, wq_np, wv_np, wp_np = [], [], [], []
    for g in range(4):
        heads = [3 * g + h for h in range(HPC)]
        wq_g = np.concatenate([w_qkv[HD * h : HD * (h + 1)] for h in heads], axis=0)
        wk_g = np.concatenate(
            [w_qkv[DIM + HD * h : DIM + HD * (h + 1)] for h in heads], axis=0
        )
        wv_g = np.concatenate(
            [w_qkv[2 * DIM + HD * h : 2 * DIM + HD * (h + 1)] for h in heads], axis=0
        )
        wq_np.append(_prep_contraction_major(np.ascontiguousarray(wq_g.T)))
        wk_np.append(_prep_contraction_major(np.ascontiguousarray(wk_g.T)))
        wv_np.append(_prep_contraction_major(np.ascontiguousarray(wv_g.T)))
        wp_np.append(
            np.ascontiguousarray(
                np.stack(
                    [w_proj[:, HD * h : HD * (h + 1)].T for h in heads], axis=0
                ).astype(NPBF16)
            )
        )

    in_maps = []
    for c in range(N_CORES):
        g, s = c >> 1, c & 1
        in_maps.append(
            {
                "xw": xw_np[s],
                "wk": wk_np[g],
                "wq": wq_np[g],
                "wv": wv_np[g],
                "wp": wp_np[g],
            }
        )
    return in_maps


def kernel(x, w_qkv, w_proj, b_proj):
    x = np.asarray(x, dtype=np.float32)
    w_qkv = np.asarray(w_qkv, dtype=np.float32)
    w_proj = np.asarray(w_proj, dtype=np.float32)
    b_proj = np.asarray(b_proj, dtype=np.float32)

    if "nc" not in _cache:
        _cache["nc"] = _build_program()
    nc = _cache["nc"]

    in_maps = prep_in_maps(x, w_qkv, w_proj)

    try:
        res = bass_utils.run_bass_kernel_spmd(
            nc,
            in_maps,
            core_ids=list(range(N_CORES)),
            trace=bool(os.environ.get("KERNEL_TRACE")),
        )
    except ModuleNotFoundError:
        # axon NTFF profiling hook unavailable in this container; rerun
        # with tracing hard-disabled.
        os.environ["BASS_NEVER_TRACE"] = "1"
        res = bass_utils.run_bass_kernel_spmd(
            nc, in_maps, core_ids=list(range(N_CORES))
        )
    kernel.last_results = res

    out = np.tile(b_proj.astype(np.float32), (N_TOK, 1))
    for c in range(N_CORES):
        s = c & 1
        out[NQ * s : NQ * (s + 1)] += res.results[c]["outp"].T.astype(np.float32)
    return out


# revision 25
# speedup vs baseline: 1.0629x; 1.0629x over previous
"""Trainium2 Bass kernel for nn_Attention (dense transformer block attention).

Full-input contract: kernel(**inputs) takes the unsharded numpy inputs and
returns the full (4096, 768) float32 output.

Sharding: 12 heads x 4096 queries -> 4 head-groups (3 heads) x 2 query
halves (2048 queries) = 8 NeuronCores. Each core computes K/V for its 3
heads over all tokens, attention for its 2048 queries, and a partial
output projection (its 192 channels of the contraction); the host sums
the 4 head-group partials per query half and adds the bias.

Per-core token order is permuted so the core's own queries are always
columns 0:2048 of xw (softmax is permutation-invariant over keys), which
removes the separate qx input.

Score matmuls are emitted as row-tiled pairs (two 64-contraction matmuls
at row groups 0/64 in adjacent instructions) so the PE array runs them
concurrently: heads 0/1 pair with each other, head 2 pairs its two
token-half layouts.
"""

import os

import numpy as np
import ml_dtypes

import concourse.bass as bass
import concourse.tile as tile
from concourse import bacc, mybir
from concourse import bass_utils

BF16 = mybir.dt.bfloat16
F32 = mybir.dt.float32
NPBF16 = ml_dtypes.bfloat16

N_TOK = 4096
DIM = 768
H = 12
HD = 64
N_CORES = 8
HPC = 3              # heads per core (head-group size)
NQ = N_TOK // 2      # queries per core
CCH = DIM // 128     # 6 contraction chunks
TCH = N_TOK // 512   # 8 token chunks of 512
KCH = N_TOK // 128   # 32 key chunks of 128
QB = NQ // 512       # 4 query blocks per core
SWEEP = 3            # exp jobs per sweep (3 PSUM banks)
SCALE = HD ** -0.5

_cache = {}


def _build_program(repeat=1):
    nc = bacc.Bacc(
        "TRN2",
        target_bir_lowering=False,
        debug=False,
        enable_asserts=False,
        num_devices=N_CORES,
    )

    xw_d = nc.dram_tensor("xw", [128, CCH, N_TOK], BF16, kind="ExternalInput").ap()
    wk_d = nc.dram_tensor("wk", [128, CCH, HPC * HD], BF16, kind="ExternalInput").ap()
    wq_d = nc.dram_tensor("wq", [128, CCH, HPC * HD], BF16, kind="ExternalInput").ap()
    wv_d = nc.dram_tensor("wv", [128, CCH, HPC * HD], BF16, kind="ExternalInput").ap()
    wp_d = nc.dram_tensor("wp", [HPC, HD, DIM], BF16, kind="ExternalInput").ap()
    out_d = nc.dram_tensor("outp", [DIM, NQ], BF16, kind="ExternalOutput").ap()

    with tile.TileContext(nc) as tc:
        with (
            tc.tile_pool(name="persist", bufs=1) as pp,
            tc.tile_pool(name="psS", bufs=2, space="PSUM") as psS,
            tc.tile_pool(name="pvp", bufs=2, space="PSUM") as pvp,
            tc.tile_pool(name="expp", bufs=12) as expp,
            tc.tile_pool(name="nrm", bufs=4) as nrm,
            tc.tile_pool(name="accp", bufs=3) as accp,
            tc.tile_pool(name="nrmd", bufs=3, space="DRAM") as nrmd,
            tc.tile_pool(name="outs", bufs=3) as outs,
        ):
            xw = pp.tile([128, CCH, N_TOK], BF16, tag="xw")
            wk = pp.tile([128, CCH, HPC * HD], BF16, tag="wk")
            wq = pp.tile([128, CCH, HPC * HD], BF16, tag="wq")
            wv = pp.tile([128, CCH, HPC * HD], BF16, tag="wv")
            wp01 = pp.tile([128, DIM], BF16, tag="wp01")
            wp2 = pp.tile([HD, DIM], BF16, tag="wp2")
            kT01 = pp.tile([128, N_TOK], BF16, tag="kT01")  # heads 0,1 stacked
            kT2ab = pp.tile([128, N_TOK // 2], BF16, tag="kT2ab")  # head 2
            qT01 = pp.tile([128, NQ], BF16, tag="qT01")
            qT2ab = pp.tile([128, NQ], BF16, tag="qT2ab")
            # V in token-major layout with a ones column appended (col HD)
            v2 = [
                pp.tile([128, KCH, HD + 1], BF16, tag=f"v2_{h}", name=f"v2_{h}")
                for h in range(HPC)
            ]
            # attention outputs: heads 0/1 stacked on partitions, head 2 alone
            attT2 = pp.tile([128, NQ], BF16, tag="attT2")
            attTh2 = pp.tile([HD, NQ], BF16, tag="attTh2")
            for h in range(HPC):
                nc.vector.memset(v2[h][:, :, HD : HD + 1], 1.0)
            onesb = pp.tile([HD + 1, HD], BF16, tag="onesb")
            nc.vector.memset(onesb, 1.0)
            ones_k = pp.tile([128, 1], BF16, tag="ones_k")
            nc.vector.memset(ones_k, 1.0)
            # stacked head-2 K/Q weights for the merged full-rate projection
            # over the query-token slices: even slices produce [K2|Q2] rows,
            # odd slices [Q2|K2] (so K2 lands in the kT2ab half it belongs).
            wkq2 = [
                pp.tile([128, CCH, 128], BF16, tag=f"wkq2_{p}", name=f"wkq2_{p}")
                for p in range(2)
            ]
            # warm the ACT exp table set before the first real sweep
            warm = pp.tile([1, 2], F32, tag="warm")
            nc.vector.memset(warm, 0.0)
            nc.scalar.activation(
                out=warm, in_=warm, func=mybir.ActivationFunctionType.Exp
            )

            counter = [0]

            def uname(p):
                counter[0] += 1
                return f"{p}{counter[0]}"

            def mm_slot():
                return psS.tile(
                    [128, SWEEP * 512], F32, tag="scores", name=uname("mmslot")
                )

            def qkv_group(w_sb, m_sl, x_sb, n_sl, out_sb, out_sl, m_par):
                ps = mm_slot()
                for c in range(CCH):
                    nc.tensor.matmul(
                        ps[0:m_par, 0:512],
                        w_sb[:, c, m_sl],
                        x_sb[:, c, n_sl],
                        start=(c == 0),
                        stop=(c == CCH - 1),
                    )
                nc.vector.tensor_copy(out=out_sb[:, out_sl], in_=ps[0:m_par, 0:512])

            def v2_group(tt, heads=(0, 1, 2)):
                tsl = slice(128 * tt, 128 * (tt + 1))
                msl = slice(HD * heads[0], HD * (heads[-1] + 1))
                ps = mm_slot()
                for c in range(CCH):
                    nc.tensor.matmul(
                        ps[:, 0 : HD * len(heads)],
                        xw[:, c, tsl],
                        wv[:, c, msl],
                        start=(c == 0),
                        stop=(c == CCH - 1),
                    )
                for i, h in enumerate(heads):
                    nc.vector.tensor_copy(
                        out=v2[h][:, tt, 0:HD],
                        in_=ps[:, HD * i : HD * (i + 1)],
                    )

            def finish_unit(h, qb, pv, fast_tail=False):
                qsl = slice(512 * qb, 512 * (qb + 1))
                pvs = nrm.tile([HD + 1, 512], F32, tag="pvs", name=uname("pvs"))
                rec = nrm.tile([HD + 1, 512], F32, tag="rec", name=uname("rec"))
                nc.vector.reciprocal(out=rec[HD : HD + 1, :], in_=pv[HD : HD + 1, :])
                nc.vector.tensor_copy(out=pvs, in_=pv)
                bc = nrm.tile([HD, 512], F32, tag="bc", name=uname("bc"))
                if fast_tail:
                    recb = nrm.tile(
                        [HD + 1, 512], BF16, tag="recb", name=uname("recb")
                    )
                    nc.vector.tensor_copy(
                        out=recb[HD : HD + 1, :], in_=rec[HD : HD + 1, :]
                    )
                    bcp = mm_slot()
                    nc.tensor.matmul(
                        bcp[0:HD, 0:512],
                        onesb[HD : HD + 1, :],
                        recb[HD : HD + 1, :],
                        start=True,
                        stop=True,
                    )
                    nc.vector.tensor_copy(out=bc, in_=bcp[0:HD, 0:512])
                else:
                    recd = nrmd.tile([1, 512], F32, tag="recd", name=uname("recd"))
                    nc.sync.dma_start(out=recd, in_=rec[HD : HD + 1, :])
                    nc.sync.dma_start(out=bc, in_=recd[:].to_broadcast((HD, 512)))
                if h < 2:
                    nc.vector.tensor_mul(
                        attT2[HD * h : HD * (h + 1), qsl], pvs[0:HD, :], bc
                    )
                else:
                    nc.vector.tensor_mul(attTh2[:, qsl], pvs[0:HD, :], bc)

            def emit_body():
                # Load order: first weights + x chunks feeding kT01/qT01, so
                # the first matmul can start ~3us in instead of one bulk DMA.
                nc.sync.dma_start(out=wk, in_=wk_d)
                nc.sync.dma_start(out=xw[:, 0:3, 0:512], in_=xw_d[:, 0:3, 0:512])
                nc.sync.dma_start(out=xw[:, 3:6, 0:512], in_=xw_d[:, 3:6, 0:512])
                nc.sync.dma_start(out=wq, in_=wq_d)
                nc.sync.dma_start(out=wv, in_=wv_d)
                for t in range(1, TCH):
                    sl = slice(512 * t, 512 * (t + 1))
                    nc.sync.dma_start(out=xw[:, :, sl], in_=xw_d[:, :, sl])
                nc.sync.dma_start(out=wp01[0:HD, :], in_=wp_d[0])
                nc.sync.dma_start(out=wp01[HD:128, :], in_=wp_d[1])
                nc.sync.dma_start(out=wp2, in_=wp_d[2])

                kT01_pending = list(range(TCH))
                qT0_pending = [0]

                def kq_jit(tokens_needed):
                    while kT01_pending and 512 * kT01_pending[0] < tokens_needed:
                        t = kT01_pending.pop(0)
                        sl = slice(512 * t, 512 * (t + 1))
                        qkv_group(wk, slice(0, 128), xw, sl, kT01, sl, 128)
                        if qT0_pending:
                            qT0_pending.pop()
                            qkv_group(
                                wq,
                                slice(0, 128),
                                xw,
                                slice(0, 512),
                                qT01,
                                slice(0, 512),
                                128,
                            )

                v2_pending = list(range(KCH))
                v2_done = [0]

                def v2_step():
                    # first unit only needs heads 0/1; head-2 V is queued as
                    # background work for the following unit.
                    if v2_pending:
                        v2_group(v2_pending.pop(0), heads=(0, 1))
                        v2_done[0] += 1

                background = []

                def kt2_pair(p):
                    # col-tiled concurrent pair: rows 0:63 <- tokens 1024p..,
                    # rows 64:127 <- +512; interleaved c-loop so the two
                    # column-group matmuls sit adjacent and overlap.
                    ps = mm_slot()
                    sl_a = slice(1024 * p, 1024 * p + 512)
                    sl_b = slice(1024 * p + 512, 1024 * p + 1024)
                    for c in range(CCH):
                        nc.tensor.matmul(
                            ps[0:HD, 0:512],
                            wk[:, c, 128 : 128 + HD],
                            xw[:, c, sl_a],
                            start=(c == 0),
                            stop=(c == CCH - 1),
                            skip_group_check=True,
                        )
                        nc.tensor.matmul(
                            ps[HD:128, 0:512],
                            wk[:, c, 128 : 128 + HD],
                            xw[:, c, sl_b],
                            start=(c == 0),
                            stop=(c == CCH - 1),
                            skip_group_check=True,
                        )
                    nc.vector.tensor_copy(
                        out=kT2ab[:, 512 * p : 512 * (p + 1)], in_=ps[:, 0:512]
                    )

                def wkq2_fill():
                    for p in range(2):
                        nc.vector.tensor_copy(
                            out=wkq2[p][:, :, 64 * p : 64 * p + 64],
                            in_=wk[:, :, 128 : 128 + HD],
                        )
                        nc.vector.tensor_copy(
                            out=wkq2[p][:, :, 64 - 64 * p : 128 - 64 * p],
                            in_=wq[:, :, 128 : 128 + HD],
                        )

                def k2q2_slice(t):
                    # merged full-rate K2+Q2 for query-token slice t (<QB):
                    # rows [K2|Q2] for even t, [Q2|K2] for odd t.
                    sl = slice(512 * t, 512 * (t + 1))
                    par = t & 1
                    ps = mm_slot()
                    for c in range(CCH):
                        nc.tensor.matmul(
                            ps[:, 0:512],
                            wkq2[par][:, c, :],
                            xw[:, c, sl],
                            start=(c == 0),
                            stop=(c == CCH - 1),
                        )
                    ksl = slice(512 * (t // 2), 512 * (t // 2) + 512)
                    nc.vector.tensor_copy(
                        out=kT2ab[HD * par : HD * (par + 1), ksl],
                        in_=ps[HD * par : HD * (par + 1), 0:512],
                    )
                    nc.vector.tensor_copy(
                        out=qT2ab[HD - HD * par : 128 - HD * par, sl],
                        in_=ps[HD - HD * par : 128 - HD * par, 0:512],
                    )

                sweep_no = [0]

                def tick_background():
                    # chunky items (kt2/qt2/proj bursts) borrow a psS slot;
                    # pace them so consecutive items can't grab both slots
                    # and stall the exp pipeline.
                    sweep_no[0] += 1
                    if background and sweep_no[0] % 2 == 0:
                        background.pop(0)()

                def drain_background():
                    while background:
                        background.pop(0)()

                def emit_pv(pv, ex, sweep_jobs, pv_start, pv_stop):
                    """unit2 path: sweep_jobs is a list of (slot_j, 2, kc);
                    pv[2] is a [HD+1, 512] bank using the ones-column trick."""
                    for j, key, kc in sweep_jobs:
                        nc.tensor.matmul(
                            pv[key],
                            v2[key][:, kc, :],
                            ex[:, 512 * j : 512 * (j + 1)],
                            start=pv_start[key],
                            stop=(kc == pv_stop[key]),
                            skip_group_check=True,
                        )
                        pv_start[key] = False

                def emit_pv01(pv, acc, ex, sweep_jobs, pv_start):
                    """unit01 path: col-tiled PV pair (h0 -> pv rows 0:64,
                    h1 -> rows 64:128, both M=64, concurrent col groups) plus
                    DVE bf16 accumulation of the softmax denominators."""
                    for j, h, kc in sweep_jobs:
                        exs = ex[:, 512 * j : 512 * (j + 1)]
                        nc.tensor.matmul(
                            pv[HD * h : HD * (h + 1), :],
                            v2[h][:, kc, 0:HD],
                            exs,
                            start=pv_start[h],
                            stop=(kc == KCH - 1),
                            skip_group_check=True,
                        )
                        pv_start[h] = False
                        if kc == 0:
                            nc.vector.tensor_copy(out=acc[h], in_=exs)
                        else:
                            nc.vector.tensor_add(acc[h], acc[h], exs)

                def finish_pair(qb, pv, acc):
                    """Normalize both heads of a unit01: partition-reduce the
                    bf16 denominator partials with a ones matmul, reciprocal,
                    broadcast back across partitions with col-tiled outer
                    products, then scale. PSUM scratch comes from pvp banks
                    so the psS sweep rotation is never blocked."""
                    qsl = slice(512 * qb, 512 * (qb + 1))
                    # copy pv out first: it frees pv's bank before den/bc
                    # rotate into it (emitting it later would deadlock the
                    # DVE FIFO against the slot-reuse WAR dependency).
                    pvs = nrm.tile([128, 512], F32, tag="pvs", name=uname("pvs"))
                    nc.vector.tensor_copy(out=pvs, in_=pv)
                    dens = [
                        pvp.tile([1, 512], F32, tag="pv", name=uname("den"))
                        for _ in range(2)
                    ]
                    bc = pvp.tile([128, 512], F32, tag="pv", name=uname("bc"))
                    recbs = []
                    for h in range(2):
                        nc.tensor.matmul(
                            dens[h], ones_k, acc[h], start=True, stop=True
                        )
                        rec = nrm.tile([1, 512], F32, tag="rec", name=uname("rec"))
                        nc.vector.reciprocal(out=rec, in_=dens[h])
                        recb = nrm.tile(
                            [1, 512], BF16, tag="recb", name=uname("recb")
                        )
                        nc.vector.tensor_copy(out=recb, in_=rec)
                        recbs.append(recb)
                    for h in range(2):
                        nc.tensor.matmul(
                            bc[HD * h : HD * (h + 1), :],
                            onesb[0:1, 0:HD],
                            recbs[h],
                            start=True,
                            stop=True,
                            skip_group_check=True,
                        )
                    nc.vector.tensor_mul(attT2[:, qsl], pvs, bc)

                def unit01(qb, first=False):
                    """Heads 0+1 for one query block: row-tiled score pairs,
                    col-tiled PV pairs, DVE denominator accumulation."""
                    qsl = slice(512 * qb, 512 * (qb + 1))
                    pv = pvp.tile([128, 512], F32, tag="pv", name=uname("pv"))
                    acc = [
                        accp.tile([128, 512], BF16, tag="acc", name=uname("acc"))
                        for _ in range(2)
                    ]
                    pv_start = {0: True, 1: True}
                    jobs = [(h, kc) for kc in range(KCH) for h in range(2)]
                    deferred = []
                    ji = 0
                    while ji < len(jobs):
                        nch = min(SWEEP, len(jobs) - ji)
                        sw = jobs[ji : ji + nch]
                        kq_jit(128 * (max(kc for _, kc in sw) + 1))
                        ps = mm_slot()
                        ex = expp.tile(
                            [128, SWEEP * 512], BF16, tag="ex", name=uname("ex")
                        )
                        for j, (h, kc) in enumerate(sw):
                            nc.tensor.matmul(
                                ps[:, 512 * j : 512 * (j + 1)],
                                kT01[HD * h : HD * (h + 1), 128 * kc : 128 * (kc + 1)],
                                qT01[HD * h : HD * (h + 1), qsl],
                                start=True,
                                stop=True,
                            )
                        nc.scalar.activation(
                            out=ex[:, 0 : 512 * nch],
                            in_=ps[:, 0 : 512 * nch],
                            func=mybir.ActivationFunctionType.Exp,
                            scale=float(SCALE),
                        )
                        sweep_jobs = [(j, h, kc) for j, (h, kc) in enumerate(sw)]
                        ji += nch
                        if first:
                            if v2_pending:
                                for _ in range(2):
                                    v2_step()
                            else:
                                tick_background()
                            deferred.append((ex, sweep_jobs))
                            while deferred and (
                                max(kc for _, _, kc in deferred[0][1])
                                < v2_done[0]
                                or len(deferred) >= 11
                            ):
                                dex, dsw = deferred.pop(0)
                                while v2_done[0] <= max(kc for _, _, kc in dsw):
                                    v2_step()
                                emit_pv01(pv, acc, dex, dsw, pv_start)
                        else:
                            emit_pv01(pv, acc, ex, sweep_jobs, pv_start)
                            tick_background()
                    if first:
                        while v2_pending:
                            v2_step()
                        for dex, dsw in deferred:
                            emit_pv01(pv, acc, dex, dsw, pv_start)
                        kq_jit(N_TOK + 1)
                    finish_pair(qb, pv, acc)

                def unit2(qb, fast_tail=False):
                    """Head 2 for one query block: the two token-half layouts
                    of kT2ab/qT2ab pair as concurrent row tiles."""
                    qsl = slice(512 * qb, 512 * (qb + 1))
                    pv = {2: pvp.tile([HD + 1, 512], F32, tag="pv", name=uname("pv"))}
                    pv_start = {2: True}
                    jobs = []
                    for idx in range(KCH // 2):
                        for half in range(2):
                            p_, i_ = idx // 4, idx % 4
                            kcg = 8 * p_ + 4 * half + i_
                            jobs.append((half, p_, i_, kcg))
                    pv_stop = {2: jobs[-1][3]}
                    ji = 0
                    while ji < len(jobs):
                        nch = min(SWEEP, len(jobs) - ji)
                        sw = jobs[ji : ji + nch]
                        ps = mm_slot()
                        ex = expp.tile(
                            [128, SWEEP * 512], BF16, tag="ex", name=uname("ex")
                        )
                        for j, (half, p_, i_, kcg) in enumerate(sw):
                            nc.tensor.matmul(
                                ps[:, 512 * j : 512 * (j + 1)],
                                kT2ab[
                                    HD * half : HD * (half + 1),
                                    512 * p_ + 128 * i_ : 512 * p_ + 128 * (i_ + 1),
                                ],
                                qT2ab[HD * half : HD * (half + 1), qsl],
                                start=True,
                                stop=True,
                            )
                        nc.scalar.activation(
                            out=ex[:, 0 : 512 * nch],
                            in_=ps[:, 0 : 512 * nch],
                            func=mybir.ActivationFunctionType.Exp,
                            scale=float(SCALE),
                        )
                        sweep_jobs = [
                            (j, 2, kcg) for j, (_, _, _, kcg) in enumerate(sw)
                        ]
                        emit_pv(pv, ex, sweep_jobs, pv_start, pv_stop)
                        ji += nch
                        tick_background()
                    finish_unit(2, qb, pv[2], fast_tail=fast_tail)

                def proj_burst(qb, half):
                    """Three 128-channel slices of the output projection in
                    one borrowed psS tile. h0+h1 contract in a single K=128
                    matmul via the stacked attT2 layout; h2 adds K=64."""
                    qsl = slice(512 * qb, 512 * (qb + 1))
                    ps = mm_slot()
                    for i in range(SWEEP):
                        m = SWEEP * half + i
                        pj = ps[:, 512 * i : 512 * (i + 1)]
                        nc.tensor.matmul(
                            pj,
                            wp01[:, 128 * m : 128 * (m + 1)],
                            attT2[:, qsl],
                            start=True,
                            stop=False,
                        )
                        nc.tensor.matmul(
                            pj,
                            wp2[:, 128 * m : 128 * (m + 1)],
                            attTh2[:, qsl],
                            start=False,
                            stop=True,
                        )
                    ot = outs.tile([128, SWEEP, 512], BF16, tag="ot", name=uname("ot"))
                    for i in range(SWEEP):
                        nc.vector.tensor_copy(
                            out=ot[:, i, :], in_=ps[:, 512 * i : 512 * (i + 1)]
                        )
                    m0 = SWEEP * half
                    nc.sync.dma_start(
                        out=out_d[128 * m0 : 128 * (m0 + SWEEP), qsl].rearrange(
                            "(a p) n -> p a n", p=128
                        ),
                        in_=ot,
                    )

                def proj_last(qb):
                    """Final query block: pack all 6 slices into psS tiles
                    (no sweeps follow, banks are free)."""
                    qsl = slice(512 * qb, 512 * (qb + 1))
                    sA = mm_slot()
                    sB = mm_slot()
                    pjs = [sA[:, 512 * i : 512 * (i + 1)] for i in range(SWEEP)]
                    pjs += [sB[:, 512 * i : 512 * (i + 1)] for i in range(SWEEP)]
                    for m in range(DIM // 128):
                        nc.tensor.matmul(
                            pjs[m],
                            wp01[:, 128 * m : 128 * (m + 1)],
                            attT2[:, qsl],
                            start=True,
                            stop=False,
                        )
                        nc.tensor.matmul(
                            pjs[m],
                            wp2[:, 128 * m : 128 * (m + 1)],
                            attTh2[:, qsl],
                            start=False,
                            stop=True,
                        )
                    for mp in range(DIM // 256):
                        ot = outs.tile(
                            [128, 2, 512], BF16, tag="ot2", name=uname("ot")
                        )
                        nc.vector.tensor_copy(out=ot[:, 0, :], in_=pjs[2 * mp])
                        nc.scalar.copy(out=ot[:, 1, :], in_=pjs[2 * mp + 1])
                        nc.sync.dma_start(
                            out=out_d[256 * mp : 256 * (mp + 1), qsl].rearrange(
                                "(a p) n -> p a n", p=128
                            ),
                            in_=ot,
                        )

                # qT01 for qb 1-3 must be ready before their units; head-2
                # K/Q (kt2/qt2) before unit2(0).
                def qt01_fill(t):
                    sl = slice(512 * t, 512 * (t + 1))
                    qkv_group(wq, slice(0, 128), xw, sl, qT01, sl, 128)

                def v2h2_quad(tt0):
                    # head-2 V for 4 key chunks in one borrowed psS bank:
                    # sequential accumulation groups, plain per-chunk copies.
                    ps = mm_slot()
                    for j in range(4):
                        for c in range(CCH):
                            nc.tensor.matmul(
                                ps[:, HD * j : HD * (j + 1)],
                                xw[:, c, 128 * (tt0 + j) : 128 * (tt0 + j + 1)],
                                wv[:, c, 2 * HD : 3 * HD],
                                start=(c == 0),
                                stop=(c == CCH - 1),
                                skip_group_check=True,
                            )
                    for j in range(4):
                        nc.vector.tensor_copy(
                            out=v2[2][:, tt0 + j, 0:HD],
                            in_=ps[:, HD * j : HD * (j + 1)],
                        )

                for t in range(1, QB):
                    background.append(lambda t=t: qt01_fill(t))
                unit01(0, first=True)
                drain_background()
                for tt0 in range(0, KCH, 4):
                    background.append(lambda tt0=tt0: v2h2_quad(tt0))
                background.append(wkq2_fill)
                for t in range(QB):
                    background.append(lambda t=t: k2q2_slice(t))
                for p in range(2, TCH // 2):
                    background.append(lambda p=p: kt2_pair(p))

                def qt2_dup():
                    # fill the missing parity half of qT2ab: even slices have
                    # Q2 in rows 64:128, odd slices in rows 0:64.
                    for t in range(QB):
                        par = t & 1
                        sl = slice(512 * t, 512 * (t + 1))
                        nc.sync.dma_start(
                            out=qT2ab[HD * par : HD * (par + 1), sl],
                            in_=qT2ab[HD - HD * par : 128 - HD * par, sl],
                        )

                background.append(qt2_dup)
                unit01(1)
                drain_background()
                unit2(0)
                background.append(lambda: proj_burst(0, 0))
                background.append(lambda: proj_burst(0, 1))
                unit01(2)
                unit2(1)
                background.append(lambda: proj_burst(1, 0))
                background.append(lambda: proj_burst(1, 1))
                unit01(3)
                unit2(2)
                background.append(lambda: proj_burst(2, 0))
                background.append(lambda: proj_burst(2, 1))
                unit2(3, fast_tail=True)
                drain_background()
                proj_last(3)

            for _rep in range(repeat):
                emit_body()

    nc.compile()
    return nc


def _prep_contraction_major(a_t: np.ndarray) -> np.ndarray:
    """(DIM, n) array -> (128, CCH, n) bf16 in SBUF layout."""
    n = a_t.shape[1]
    return np.ascontiguousarray(
        a_t.reshape(CCH, 128, n).transpose(1, 0, 2).astype(NPBF16)
    )


def prep_in_maps(x, w_qkv, w_proj):
    """Build the 8 per-core input dicts from the full-precision inputs."""
    # Token permutation per query half: own queries first.
    xw_np = [
        _prep_contraction_major(
            np.ascontiguousarray(
                (x if s == 0 else np.concatenate([x[NQ:], x[:NQ]])).T
            )
        )
        for s in range(2)
    ]

    wk_np, wq_np, wv_np, wp_np = [], [], [], []
    for g in range(4):
        heads = [3 * g + h for h in range(HPC)]
        wq_g = np.concatenate([w_qkv[HD * h : HD * (h + 1)] for h in heads], axis=0)
        wk_g = np.concatenate(
            [w_qkv[DIM + HD * h : DIM + HD * (h + 1)] for h in heads], axis=0
        )
        wv_g = np.concatenate(
            [w_qkv[2 * DIM + HD * h : 2 * DIM + HD * (h + 1)] for h in heads], axis=0
        )
        wq_np.append(_prep_contraction_major(np.ascontiguousarray(wq_g.T)))
        wk_np.append(_prep_contraction_major(np.ascontiguousarray(wk_g.T)))
        wv_np.append(_prep_contraction_major(np.ascontiguousarray(wv_g.T)))
        wp_np.append(
            np.ascontiguousarray(
                np.stack(
                    [w_proj[:, HD * h : HD * (h + 1)].T for h in heads], axis=0
                ).astype(NPBF16)
            )
        )

    in_maps = []
    for c in range(N_CORES):
        g, s = c >> 1, c & 1
        in_maps.append(
            {
                "xw": xw_np[s],
                "wk": wk_np[g],
                "wq": wq_np[g],
                "wv": wv_np[g],
                "wp": wp_np[g],
            }
        )
    return in_maps


def kernel(x, w_qkv, w_proj, b_proj):
    x = np.asarray(x, dtype=np.float32)
    w_qkv = np.asarray(w_qkv, dtype=np.float32)
    w_proj = np.asarray(w_proj, dtype=np.float32)
    b_proj = np.asarray(b_proj, dtype=np.float32)

    if "nc" not in _cache:
        _cache["nc"] = _build_program()
    nc = _cache["nc"]

    in_maps = prep_in_maps(x, w_qkv, w_proj)

    try:
        res = bass_utils.run_bass_kernel_spmd(
            nc,
            in_maps,
            core_ids=list(range(N_CORES)),
            trace=bool(os.environ.get("KERNEL_TRACE")),
        )
    except ModuleNotFoundError:
        # axon NTFF profiling hook unavailable in this container; rerun
        # with tracing hard-disabled.
        os.environ["BASS_NEVER_TRACE"] = "1"
        res = bass_utils.run_bass_kernel_spmd(
            nc, in_maps, core_ids=list(range(N_CORES))
        )
    kernel.last_results = res

    out = np.tile(b_proj.astype(np.float32), (N_TOK, 1))
    for c in range(N_CORES):
        s = c & 1
        out[NQ * s : NQ * (s + 1)] += res.results[c]["outp"].T.astype(np.float32)
    return out


# revision 48
# speedup vs baseline: 1.2670x; 1.1919x over previous
"""Trainium2 Bass kernel for nn_Attention (dense transformer block attention).

Full-input contract: kernel(**inputs) takes the unsharded numpy inputs and
returns the full (4096, 768) float32 output.

Sharding: 12 heads x 4096 queries -> 4 head-groups (3 heads) x 2 query
halves (2048 queries) = 8 NeuronCores. Each core computes K/V for its 3
heads over all tokens, attention for its 2048 queries, and a partial
output projection (its 192 channels of the contraction); the host sums
the 4 head-group partials per query half and adds the bias.

Per-core token order is permuted so the core's own queries are always
columns 0:2048 of xw (softmax is permutation-invariant over keys), which
removes the separate qx input.

Score matmuls are emitted as row-tiled pairs (two 64-contraction matmuls
at row groups 0/64 in adjacent instructions) so the PE array runs them
concurrently: heads 0/1 pair with each other, head 2 pairs its two
token-half layouts.
"""

import os

import numpy as np
import ml_dtypes

import concourse.bass as bass
import concourse.tile as tile
from concourse import bacc, mybir
from concourse import bass_utils

BF16 = mybir.dt.bfloat16
F32 = mybir.dt.float32
NPBF16 = ml_dtypes.bfloat16

N_TOK = 4096
DIM = 768
H = 12
HD = 64
N_CORES = 8
HPC = 3              # heads per core (head-group size)
NQ = N_TOK // 2      # queries per core
CCH = DIM // 128     # 6 contraction chunks
TCH = N_TOK // 512   # 8 token chunks of 512
KCH = N_TOK // 128   # 32 key chunks of 128
QB = NQ // 512       # 4 query blocks per core
SWEEP = 3            # exp jobs per sweep (3 PSUM banks)
SCALE = HD ** -0.5

_cache = {}


def _build_program(repeat=1):
    nc = bacc.Bacc(
        "TRN2",
        target_bir_lowering=False,
        debug=False,
        enable_asserts=False,
        num_devices=N_CORES,
    )

    xw_d = nc.dram_tensor("xw", [128, CCH, N_TOK], BF16, kind="ExternalInput").ap()
    wk_d = nc.dram_tensor("wk", [128, CCH, HPC * HD], BF16, kind="ExternalInput").ap()
    wq_d = nc.dram_tensor("wq", [128, CCH, HPC * HD], BF16, kind="ExternalInput").ap()
    wv_d = nc.dram_tensor("wv", [128, CCH, HPC * HD], BF16, kind="ExternalInput").ap()
    wp_d = nc.dram_tensor("wp", [HPC, HD, DIM], BF16, kind="ExternalInput").ap()
    out_d = nc.dram_tensor("outp", [DIM, NQ], BF16, kind="ExternalOutput").ap()

    with tile.TileContext(nc) as tc:
        with (
            tc.tile_pool(name="persist", bufs=1) as pp,
            tc.tile_pool(name="psS", bufs=2, space="PSUM") as psS,
            tc.tile_pool(name="pvp", bufs=2, space="PSUM") as pvp,
            tc.tile_pool(name="expp", bufs=12) as expp,
            tc.tile_pool(name="nrm", bufs=4) as nrm,
            tc.tile_pool(name="accp", bufs=2) as accp,
            tc.tile_pool(name="nrmd", bufs=3, space="DRAM") as nrmd,
            tc.tile_pool(name="outs", bufs=2) as outs,
        ):
            xw = pp.tile([128, CCH, N_TOK], BF16, tag="xw")
            wk = pp.tile([128, CCH, HPC * HD], BF16, tag="wk")
            wq = pp.tile([128, CCH, HPC * HD], BF16, tag="wq")
            wv = pp.tile([128, CCH, HPC * HD], BF16, tag="wv")
            wp01 = pp.tile([128, DIM], BF16, tag="wp01")
            wp2 = pp.tile([HD, DIM], BF16, tag="wp2")
            kT01 = pp.tile([128, N_TOK], BF16, tag="kT01")  # heads 0,1 stacked
            kT2ab = pp.tile([128, N_TOK // 2], BF16, tag="kT2ab")  # head 2
            qT01 = pp.tile([128, NQ], BF16, tag="qT01")
            qT2ab = pp.tile([128, NQ], BF16, tag="qT2ab")
            # V in token-major layout; heads 0/1 have no ones column (their
            # denominators come from the DVE accumulator), head 2 keeps the
            # ones-column trick.
            v2 = [
                pp.tile([128, KCH, HD], BF16, tag=f"v2_{h}", name=f"v2_{h}")
                for h in range(2)
            ]
            v2h2 = pp.tile([128, KCH, HD + 1], BF16, tag="v2h2")
            # attention outputs: heads 0/1 stacked on partitions, head 2 alone
            attT2 = pp.tile([128, NQ], BF16, tag="attT2")
            attTh2 = pp.tile([HD, NQ], BF16, tag="attTh2")
            nc.vector.memset(v2h2[:, :, HD : HD + 1], 1.0)
            onesb = pp.tile([HD + 1, HD], BF16, tag="onesb")
            nc.vector.memset(onesb, 1.0)
            ones_k = pp.tile([128, 1], BF16, tag="ones_k")
            nc.vector.memset(ones_k, 1.0)
            # stacked head-2 K/Q weights for the merged full-rate projection
            # over the query-token slices: even slices produce [K2|Q2] rows,
            # odd slices [Q2|K2] (so K2 lands in the kT2ab half it belongs).
            wkq2 = [
                pp.tile([128, CCH, 128], BF16, tag=f"wkq2_{p}", name=f"wkq2_{p}")
                for p in range(2)
            ]
            # warm the ACT exp table set before the first real sweep
            warm = pp.tile([1, 2], F32, tag="warm")
            nc.vector.memset(warm, 0.0)
            nc.scalar.activation(
                out=warm, in_=warm, func=mybir.ActivationFunctionType.Exp
            )

            counter = [0]

            def uname(p):
                counter[0] += 1
                return f"{p}{counter[0]}"

            def mm_slot():
                return psS.tile(
                    [128, SWEEP * 512], F32, tag="scores", name=uname("mmslot")
                )

            def qkv_group(w_sb, m_sl, x_sb, n_sl, out_sb, out_sl, m_par):
                ps = mm_slot()
                for c in range(CCH):
                    nc.tensor.matmul(
                        ps[0:m_par, 0:512],
                        w_sb[:, c, m_sl],
                        x_sb[:, c, n_sl],
                        start=(c == 0),
                        stop=(c == CCH - 1),
                    )
                nc.vector.tensor_copy(out=out_sb[:, out_sl], in_=ps[0:m_par, 0:512])

            def v2_group(tt):
                """token-major V for heads 0/1, one 128-token key chunk."""
                tsl = slice(128 * tt, 128 * (tt + 1))
                ps = psS.tile([128, SWEEP * 512], F32, tag="scores", name=f"v2g{tt}")
                for c in range(CCH):
                    nc.tensor.matmul(
                        ps[:, 0:128],
                        xw[:, c, tsl],
                        wv[:, c, 0:128],
                        start=(c == 0),
                        stop=(c == CCH - 1),
                    )
                for h in range(2):
                    nc.vector.tensor_copy(
                        out=v2[h][:, tt, :],
                        in_=ps[:, HD * h : HD * (h + 1)],
                    )

            def finish_unit(h, qb, pv, fast_tail=False):
                qsl = slice(512 * qb, 512 * (qb + 1))
                pvs = nrm.tile([HD + 1, 512], F32, tag="pvs", name=uname("pvs"))
                rec = nrm.tile([HD + 1, 512], F32, tag="rec", name=uname("rec"))
                nc.vector.reciprocal(out=rec[HD : HD + 1, :], in_=pv[HD : HD + 1, :])
                nc.vector.tensor_copy(out=pvs, in_=pv)
                bc = nrm.tile([HD, 512], F32, tag="bc", name=uname("bc"))
                if fast_tail:
                    recb = nrm.tile(
                        [HD + 1, 512], BF16, tag="recb", name=uname("recb")
                    )
                    nc.vector.tensor_copy(
                        out=recb[HD : HD + 1, :], in_=rec[HD : HD + 1, :]
                    )
                    bcp = mm_slot()
                    nc.tensor.matmul(
                        bcp[0:HD, 0:512],
                        onesb[HD : HD + 1, :],
                        recb[HD : HD + 1, :],
                        start=True,
                        stop=True,
                    )
                    nc.vector.tensor_copy(out=bc, in_=bcp[0:HD, 0:512])
                else:
                    recd = nrmd.tile([1, 512], F32, tag="recd", name=uname("recd"))
                    nc.sync.dma_start(out=recd, in_=rec[HD : HD + 1, :])
                    nc.sync.dma_start(out=bc, in_=recd[:].to_broadcast((HD, 512)))
                if h < 2:
                    nc.vector.tensor_mul(
                        attT2[HD * h : HD * (h + 1), qsl], pvs[0:HD, :], bc
                    )
                else:
                    nc.vector.tensor_mul(attTh2[:, qsl], pvs[0:HD, :], bc)

            def emit_body():
                # Load order: first weights + x chunks feeding kT01/qT01, so
                # the first matmul can start ~3us in instead of one bulk DMA.
                nc.sync.dma_start(out=wk, in_=wk_d)
                nc.sync.dma_start(out=xw[:, 0:3, 0:512], in_=xw_d[:, 0:3, 0:512])
                nc.sync.dma_start(out=xw[:, 3:6, 0:512], in_=xw_d[:, 3:6, 0:512])
                nc.sync.dma_start(out=wq, in_=wq_d)
                nc.sync.dma_start(out=wv, in_=wv_d)
                for t in range(1, TCH):
                    sl = slice(512 * t, 512 * (t + 1))
                    nc.sync.dma_start(out=xw[:, :, sl], in_=xw_d[:, :, sl])
                nc.sync.dma_start(out=wp01[0:HD, :], in_=wp_d[0])
                nc.sync.dma_start(out=wp01[HD:128, :], in_=wp_d[1])
                nc.sync.dma_start(out=wp2, in_=wp_d[2])

                kT01_pending = list(range(TCH))
                qT0_pending = [0]

                def kq_jit(tokens_needed):
                    while kT01_pending and 512 * kT01_pending[0] < tokens_needed:
                        t = kT01_pending.pop(0)
                        sl = slice(512 * t, 512 * (t + 1))
                        qkv_group(wk, slice(0, 128), xw, sl, kT01, sl, 128)
                        if qT0_pending:
                            qT0_pending.pop()
                            qkv_group(
                                wq,
                                slice(0, 128),
                                xw,
                                slice(0, 512),
                                qT01,
                                slice(0, 512),
                                128,
                            )

                v2_pending = list(range(KCH))
                v2_done = [0]  # counts completed 128-token key chunks

                def v2_step():
                    # first unit only needs heads 0/1; head-2 V is queued as
                    # background work for the following unit.
                    if v2_pending:
                        v2_group(v2_pending.pop(0))
                        v2_done[0] += 1

                background = []

                def kt2_pair(p):
                    # col-tiled concurrent pair: rows 0:63 <- tokens 1024p..,
                    # rows 64:127 <- +512; interleaved c-loop so the two
                    # column-group matmuls sit adjacent and overlap.
                    ps = mm_slot()
                    sl_a = slice(1024 * p, 1024 * p + 512)
                    sl_b = slice(1024 * p + 512, 1024 * p + 1024)
                    for c in range(CCH):
                        nc.tensor.matmul(
                            ps[0:HD, 0:512],
                            wk[:, c, 128 : 128 + HD],
                            xw[:, c, sl_a],
                            start=(c == 0),
                            stop=(c == CCH - 1),
                            skip_group_check=True,
                        )
                        nc.tensor.matmul(
                            ps[HD:128, 0:512],
                            wk[:, c, 128 : 128 + HD],
                            xw[:, c, sl_b],
                            start=(c == 0),
                            stop=(c == CCH - 1),
                            skip_group_check=True,
                        )
                    nc.vector.tensor_copy(
                        out=kT2ab[:, 512 * p : 512 * (p + 1)], in_=ps[:, 0:512]
                    )

                def wkq2_fill():
                    for p in range(2):
                        nc.vector.tensor_copy(
                            out=wkq2[p][:, :, 64 * p : 64 * p + 64],
                            in_=wk[:, :, 128 : 128 + HD],
                        )
                        nc.vector.tensor_copy(
                            out=wkq2[p][:, :, 64 - 64 * p : 128 - 64 * p],
                            in_=wq[:, :, 128 : 128 + HD],
                        )

                def k2q2_slice(t):
                    # merged full-rate K2+Q2 for query-token slice t (<QB):
                    # rows [K2|Q2] for even t, [Q2|K2] for odd t.
                    sl = slice(512 * t, 512 * (t + 1))
                    par = t & 1
                    ps = mm_slot()
                    for c in range(CCH):
                        nc.tensor.matmul(
                            ps[:, 0:512],
                            wkq2[par][:, c, :],
                            xw[:, c, sl],
                            start=(c == 0),
                            stop=(c == CCH - 1),
                        )
                    ksl = slice(512 * (t // 2), 512 * (t // 2) + 512)
                    nc.vector.tensor_copy(
                        out=kT2ab[HD * par : HD * (par + 1), ksl],
                        in_=ps[HD * par : HD * (par + 1), 0:512],
                    )
                    nc.vector.tensor_copy(
                        out=qT2ab[HD - HD * par : 128 - HD * par, sl],
                        in_=ps[HD - HD * par : 128 - HD * par, 0:512],
                    )

                def tick_background():
                    # one item per sweep: the psS slot rotation alternates
                    # between sweeps and items, which is exactly the normal
                    # double-buffer cadence.
                    if background:
                        background.pop(0)()

                def drain_background():
                    while background:
                        background.pop(0)()

                def emit_pv(pv, ex, sweep_jobs, pv_start, pv_stop):
                    """unit2 path: sweep_jobs is a list of (slot_j, 2, kc);
                    pv[2] is a [HD+1, 512] bank using the ones-column trick."""
                    for j, key, kc in sweep_jobs:
                        nc.tensor.matmul(
                            pv[key],
                            v2h2[:, kc, :],
                            ex[:, 512 * j : 512 * (j + 1)],
                            start=pv_start[key],
                            stop=(kc == pv_stop[key]),
                            skip_group_check=True,
                        )
                        pv_start[key] = False

                def emit_pv01(pv, ex, sweep_jobs, pv_start):
                    """unit01 path: col-tiled PV pair (h0 -> pv rows 0:64,
                    h1 -> rows 64:128, both M=64, concurrent col groups)."""
                    for j, h, kc in sweep_jobs:
                        nc.tensor.matmul(
                            pv[HD * h : HD * (h + 1), :],
                            v2[h][:, kc, :],
                            ex[:, 512 * j : 512 * (j + 1)],
                            start=pv_start[h],
                            stop=(kc == KCH - 1),
                            skip_group_check=True,
                        )
                        pv_start[h] = False

                def finish_pair(qb, pv, acc6):
                    """Normalize both heads of a unit01: partition-reduce the
                    bf16 denominator partials with a ones matmul, reciprocal,
                    broadcast back across partitions with col-tiled outer
                    products, then scale. PSUM scratch comes from pvp banks
                    so the psS sweep rotation is never blocked."""
                    qsl = slice(512 * qb, 512 * (qb + 1))
                    # copy pv out first: it frees pv's bank before den/bc
                    # rotate into it (emitting it later would deadlock the
                    # DVE FIFO against the slot-reuse WAR dependency).
                    pvs = nrm.tile([128, 512], F32, tag="pvs", name=uname("pvs"))
                    nc.vector.tensor_copy(out=pvs, in_=pv)
                    dens = [
                        pvp.tile([1, 512], F32, tag="pv", name=uname("den"))
                        for _ in range(2)
                    ]
                    bc = pvp.tile([128, 512], F32, tag="pv", name=uname("bc"))
                    recbs = []
                    for h in range(2):
                        for i, p in enumerate((h, h + 2, h + 4)):
                            nc.tensor.matmul(
                                dens[h],
                                ones_k,
                                acc6[:, 512 * p : 512 * (p + 1)],
                                start=(i == 0),
                                stop=(i == 2),
                            )
                        rec = nrm.tile([1, 512], F32, tag="rec", name=uname("rec"))
                        nc.vector.reciprocal(out=rec, in_=dens[h])
                        recb = nrm.tile(
                            [1, 512], BF16, tag="recb", name=uname("recb")
                        )
                        nc.vector.tensor_copy(out=recb, in_=rec)
                        recbs.append(recb)
                    for h in range(2):
                        nc.tensor.matmul(
                            bc[HD * h : HD * (h + 1), :],
                            onesb[0:1, 0:HD],
                            recbs[h],
                            start=True,
                            stop=True,
                            skip_group_check=True,
                        )
                    nc.vector.tensor_mul(attT2[:, qsl], pvs, bc)

                def unit01(qb, first=False):
                    """Heads 0+1 for one query block: row-tiled score pairs,
                    col-tiled PV pairs. Denominators accumulate in a 6-slot
                    bf16 buffer with ONE wide DVE add per sweep (job j has
                    head j%2, so slot j%6 is always the same head)."""
                    qsl = slice(512 * qb, 512 * (qb + 1))
                    pv = pvp.tile([128, 512], F32, tag="pv", name=uname("pv"))
                    acc6 = accp.tile(
                        [128, 2 * SWEEP * 512], BF16, tag="acc", name=uname("acc")
                    )
                    pv_start = {0: True, 1: True}
                    jobs = [(h, kc) for kc in range(KCH) for h in range(2)]
                    deferred = []
                    ji = 0
                    sweep_idx = 0
                    while ji < len(jobs):
                        nch = min(SWEEP, len(jobs) - ji)
                        sw = jobs[ji : ji + nch]
                        kq_jit(128 * (max(kc for _, kc in sw) + 1))
                        ps = mm_slot()
                        ex = expp.tile(
                            [128, SWEEP * 512], BF16, tag="ex", name=uname("ex")
                        )
                        for j, (h, kc) in enumerate(sw):
                            nc.tensor.matmul(
                                ps[:, 512 * j : 512 * (j + 1)],
                                kT01[HD * h : HD * (h + 1), 128 * kc : 128 * (kc + 1)],
                                qT01[HD * h : HD * (h + 1), qsl],
                                start=True,
                                stop=True,
                            )
                        nc.scalar.activation(
                            out=ex[:, 0 : 512 * nch],
                            in_=ps[:, 0 : 512 * nch],
                            func=mybir.ActivationFunctionType.Exp,
                            scale=float(SCALE),
                        )
                        a0 = 512 * SWEEP * (sweep_idx & 1)
                        asl = ex[:, 0 : 512 * nch]
                        dst = acc6[:, a0 : a0 + 512 * nch]
                        if sweep_idx < 2:
                            nc.vector.tensor_copy(out=dst, in_=asl)
                        else:
                            nc.vector.tensor_add(dst, dst, asl)
                        sweep_jobs = [(j, h, kc) for j, (h, kc) in enumerate(sw)]
                        ji += nch
                        sweep_idx += 1
                        if first:
                            if v2_pending:
                                for _ in range(2):
                                    v2_step()
                            else:
                                tick_background()
                            deferred.append((ex, sweep_jobs))
                            while deferred and (
                                max(kc for _, _, kc in deferred[0][1])
                                < v2_done[0]
                                or len(deferred) >= 11
                            ):
                                dex, dsw = deferred.pop(0)
                                while v2_done[0] <= max(kc for _, _, kc in dsw):
                                    v2_step()
                                emit_pv01(pv, dex, dsw, pv_start)
                        else:
                            emit_pv01(pv, ex, sweep_jobs, pv_start)
                            tick_background()
                    if first:
                        while v2_pending:
                            v2_step()
                        for dex, dsw in deferred:
                            emit_pv01(pv, dex, dsw, pv_start)
                        kq_jit(N_TOK + 1)
                    finish_pair(qb, pv, acc6)

                def unit2(qb, fast_tail=False):
                    """Head 2 for one query block: the two token-half layouts
                    of kT2ab/qT2ab pair as concurrent row tiles."""
                    qsl = slice(512 * qb, 512 * (qb + 1))
                    pv = {2: pvp.tile([HD + 1, 512], F32, tag="pv", name=uname("pv"))}
                    pv_start = {2: True}
                    jobs = []
                    for idx in range(KCH // 2):
                        for half in range(2):
                            p_, i_ = idx // 4, idx % 4
                            kcg = 8 * p_ + 4 * half + i_
                            jobs.append((half, p_, i_, kcg))
                    pv_stop = {2: jobs[-1][3]}
                    ji = 0
                    while ji < len(jobs):
                        nch = min(SWEEP, len(jobs) - ji)
                        sw = jobs[ji : ji + nch]
                        ps = mm_slot()
                        ex = expp.tile(
                            [128, SWEEP * 512], BF16, tag="ex", name=uname("ex")
                        )
                        for j, (half, p_, i_, kcg) in enumerate(sw):
                            nc.tensor.matmul(
                                ps[:, 512 * j : 512 * (j + 1)],
                                kT2ab[
                                    HD * half : HD * (half + 1),
                                    512 * p_ + 128 * i_ : 512 * p_ + 128 * (i_ + 1),
                                ],
                                qT2ab[HD * half : HD * (half + 1), qsl],
                                start=True,
                                stop=True,
                            )
                        nc.scalar.activation(
                            out=ex[:, 0 : 512 * nch],
                            in_=ps[:, 0 : 512 * nch],
                            func=mybir.ActivationFunctionType.Exp,
                            scale=float(SCALE),
                        )
                        sweep_jobs = [
                            (j, 2, kcg) for j, (_, _, _, kcg) in enumerate(sw)
                        ]
                        emit_pv(pv, ex, sweep_jobs, pv_start, pv_stop)
                        ji += nch
                        tick_background()
                    finish_unit(2, qb, pv[2], fast_tail=fast_tail)

                def proj_burst(qb, half):
                    """Three 128-channel slices of the output projection in
                    one borrowed psS tile. h0+h1 contract in a single K=128
                    matmul via the stacked attT2 layout; h2 adds K=64."""
                    qsl = slice(512 * qb, 512 * (qb + 1))
                    ps = mm_slot()
                    for i in range(SWEEP):
                        m = SWEEP * half + i
                        pj = ps[:, 512 * i : 512 * (i + 1)]
                        nc.tensor.matmul(
                            pj,
                            wp01[:, 128 * m : 128 * (m + 1)],
                            attT2[:, qsl],
                            start=True,
                            stop=False,
                        )
                        nc.tensor.matmul(
                            pj,
                            wp2[:, 128 * m : 128 * (m + 1)],
                            attTh2[:, qsl],
                            start=False,
                            stop=True,
                        )
                    ot = outs.tile([128, SWEEP, 512], BF16, tag="ot", name=uname("ot"))
                    for i in range(SWEEP):
                        nc.vector.tensor_copy(
                            out=ot[:, i, :], in_=ps[:, 512 * i : 512 * (i + 1)]
                        )
                    m0 = SWEEP * half
                    nc.sync.dma_start(
                        out=out_d[128 * m0 : 128 * (m0 + SWEEP), qsl].rearrange(
                            "(a p) n -> p a n", p=128
                        ),
                        in_=ot,
                    )

                def proj_last(qb):
                    """Final query block: pack all 6 slices into psS tiles
                    (no sweeps follow, banks are free)."""
                    qsl = slice(512 * qb, 512 * (qb + 1))
                    sA = mm_slot()
                    sB = mm_slot()
                    pjs = [sA[:, 512 * i : 512 * (i + 1)] for i in range(SWEEP)]
                    pjs += [sB[:, 512 * i : 512 * (i + 1)] for i in range(SWEEP)]
                    for m in range(DIM // 128):
                        nc.tensor.matmul(
                            pjs[m],
                            wp01[:, 128 * m : 128 * (m + 1)],
                            attT2[:, qsl],
                            start=True,
                            stop=False,
                        )
                        nc.tensor.matmul(
                            pjs[m],
                            wp2[:, 128 * m : 128 * (m + 1)],
                            attTh2[:, qsl],
                            start=False,
                            stop=True,
                        )
                    for mp in range(DIM // 256):
                        ot = outs.tile(
                            [128, 2, 512], BF16, tag="ot2", name=uname("ot")
                        )
                        nc.vector.tensor_copy(out=ot[:, 0, :], in_=pjs[2 * mp])
                        nc.scalar.copy(out=ot[:, 1, :], in_=pjs[2 * mp + 1])
                        nc.sync.dma_start(
                            out=out_d[256 * mp : 256 * (mp + 1), qsl].rearrange(
                                "(a p) n -> p a n", p=128
                            ),
                            in_=ot,
                        )

                # qT01 for qb 1-3 must be ready before their units; head-2
                # K/Q (kt2/qt2) before unit2(0).
                def qt01_fill(t):
                    sl = slice(512 * t, 512 * (t + 1))
                    qkv_group(wq, slice(0, 128), xw, sl, qT01, sl, 128)

                for t in range(1, QB):
                    background.append(lambda t=t: qt01_fill(t))
                def v2h2_quad(tt0):
                    # head-2 V for 4 key chunks in one borrowed psS bank:
                    # sequential accumulation groups, plain per-chunk copies.
                    ps = mm_slot()
                    for j in range(4):
                        for c in range(CCH):
                            nc.tensor.matmul(
                                ps[:, HD * j : HD * (j + 1)],
                                xw[:, c, 128 * (tt0 + j) : 128 * (tt0 + j + 1)],
                                wv[:, c, 2 * HD : 3 * HD],
                                start=(c == 0),
                                stop=(c == CCH - 1),
                                skip_group_check=True,
                            )
                    for j in range(4):
                        nc.vector.tensor_copy(
                            out=v2h2[:, tt0 + j, 0:HD],
                            in_=ps[:, HD * j : HD * (j + 1)],
                        )

                unit01(0, first=True)
                drain_background()
                # U2(0)'s score operands (kT2ab/qT2ab) first, then its PV
                # operands (v2h2).
                background.append(wkq2_fill)
                for t in range(QB):
                    background.append(lambda t=t: k2q2_slice(t))
                def qt2_dup():
                    # fill the missing parity half of qT2ab: even slices have
                    # Q2 in rows 64:128, odd slices in rows 0:64.
                    for t in range(QB):
                        par = t & 1
                        sl = slice(512 * t, 512 * (t + 1))
                        nc.sync.dma_start(
                            out=qT2ab[HD * par : HD * (par + 1), sl],
                            in_=qT2ab[HD - HD * par : 128 - HD * par, sl],
                        )

                background.append(qt2_dup)
                for p in range(2, TCH // 2):
                    background.append(lambda p=p: kt2_pair(p))
                for tt0 in range(0, KCH, 4):
                    background.append(lambda tt0=tt0: v2h2_quad(tt0))
                unit01(1)
                drain_background()
                unit2(0)
                background.append(lambda: proj_burst(0, 0))
                background.append(lambda: proj_burst(0, 1))
                unit01(2)
                unit2(1)
                background.append(lambda: proj_burst(1, 0))
                background.append(lambda: proj_burst(1, 1))
                unit01(3)
                unit2(2)
                background.append(lambda: proj_burst(2, 0))
                background.append(lambda: proj_burst(2, 1))
                unit2(3, fast_tail=True)
                drain_background()
                proj_last(3)

            for _rep in range(repeat):
                emit_body()

    nc.compile()
    return nc


def _prep_contraction_major(a_t: np.ndarray) -> np.ndarray:
    """(DIM, n) array -> (128, CCH, n) bf16 in SBUF layout."""
    n = a_t.shape[1]
    return np.ascontiguousarray(
        a_t.reshape(CCH, 128, n).transpose(1, 0, 2).astype(NPBF16)
    )


def prep_in_maps(x, w_qkv, w_proj):
    """Build the 8 per-core input dicts from the full-precision inputs."""
    # Token permutation per query half: own queries first.
    xw_np = [
        _prep_contraction_major(
            np.ascontiguousarray(
                (x if s == 0 else np.concatenate([x[NQ:], x[:NQ]])).T
            )
        )
        for s in range(2)
    ]

    wk_np, wq_np, wv_np, wp_np = [], [], [], []
    for g in range(4):
        heads = [3 * g + h for h in range(HPC)]
        wq_g = np.concatenate([w_qkv[HD * h : HD * (h + 1)] for h in heads], axis=0)
        wk_g = np.concatenate(
            [w_qkv[DIM + HD * h : DIM + HD * (h + 1)] for h in heads], axis=0
        )
        wv_g = np.concatenate(
            [w_qkv[2 * DIM + HD * h : 2 * DIM + HD * (h + 1)] for h in heads], axis=0
        )
        wq_np.append(_prep_contraction_major(np.ascontiguousarray(wq_g.T)))
        wk_np.append(_prep_contraction_major(np.ascontiguousarray(wk_g.T)))
        wv_np.append(_prep_contraction_major(np.ascontiguousarray(wv_g.T)))
        wp_np.append(
            np.ascontiguousarray(
                np.stack(
                    [w_proj[:, HD * h : HD * (h + 1)].T for h in heads], axis=0
                ).astype(NPBF16)
            )
        )

    in_maps = []
    for c in range(N_CORES):
        g, s = c >> 1, c & 1
        in_maps.append(
            {
                "xw": xw_np[s],
                "wk": wk_np[g],
                "wq": wq_np[g],
                "wv": wv_np[g],
                "wp": wp_np[g],
            }
        )
    return in_maps


def kernel(x, w_qkv, w_proj, b_proj):
    x = np.asarray(x, dtype=np.float32)
    w_qkv = np.asarray(w_qkv, dtype=np.float32)
    w_proj = np.asarray(w_proj, dtype=np.float32)
    b_proj = np.asarray(b_proj, dtype=np.float32)

    if "nc" not in _cache:
        _cache["nc"] = _build_program()
    nc = _cache["nc"]

    in_maps = prep_in_maps(x, w_qkv, w_proj)

    try:
        res = bass_utils.run_bass_kernel_spmd(
            nc,
            in_maps,
            core_ids=list(range(N_CORES)),
            trace=bool(os.environ.get("KERNEL_TRACE")),
        )
    except ModuleNotFoundError:
        # axon NTFF profiling hook unavailable in this container; rerun
        # with tracing hard-disabled.
        os.environ["BASS_NEVER_TRACE"] = "1"
        res = bass_utils.run_bass_kernel_spmd(
            nc, in_maps, core_ids=list(range(N_CORES))
        )
    kernel.last_results = res

    out = np.tile(b_proj.astype(np.float32), (N_TOK, 1))
    for c in range(N_CORES):
        s = c & 1
        out[NQ * s : NQ * (s + 1)] += res.results[c]["outp"].T.astype(np.float32)
    return out
